# revision 16
# baseline (speedup 1.0000x reference)
"""Trainium2 Bass kernel for nn_Attention_22454089023887 (sparse_attention).

LayerNorm -> QKV -> 8-head attention with gathered rel-pos bias -> softmax -> proj.
Sharding: data-parallel over batch B=32 across 8 cores (4 batches/core), no
collectives.  The bias gather attn_biases[:, bias_idxs] has no efficient device
primitive (GPSIMD gather is ~100x too slow for 8M elements), so exp(bias) is
precomputed host-side and streamed as a bf16 input; the softmax applies it
multiplicatively: exp(s + b) = exp(s) * exp(b).

Device-side layout choices (see comments inline):
  - scores are computed transposed, ST[m, n], so the AV matmul can contract
    over m on partitions with no attention-matrix transpose;
  - softmax Z comes free from a ones-column appended to V (row 64 of AV psum);
  - all LN/QKV bias terms that are softmax-row-constants are dropped; the
    m-varying term scale*bq.k0[m] enters as an extra QKV output channel;
  - v-bias and beta/gamma fold into host-side weight preprocessing.
"""

import sys
import os

sys.path.insert(0, "/opt/trn_rl_repo")

import numpy as np
import ml_dtypes

B, N, DIM = 32, 1024, 256
H, KD, AR = 8, 16, 4
D = AR * KD  # 64
DH = D * H  # 512
SCALE = KD ** (-0.5)  # 0.25
EPS = 1e-5
NCORES = 8
BL = B // NCORES  # 4 batches per core
T = BL * N  # 4096 tokens per core
BH = BL * H  # 32 (b, h) pairs per core

_BF16 = ml_dtypes.bfloat16

_CACHE = {}


def _build():
    """Build the Bass graph once; returns (nc, names) for run_bass_kernel_spmd."""
    import concourse.bass as bass
    import concourse.tile as tile
    from concourse import bacc, mybir
    from concourse.masks import make_identity

    f32 = mybir.dt.float32
    bf16 = mybir.dt.bfloat16
    AF = mybir.ActivationFunctionType
    ALU = mybir.AluOpType

    nc = bacc.Bacc("TRN2", target_bir_lowering=False, debug=False,
                   num_devices=NCORES)

    # ---- DRAM parameters (per-core shards / replicated tables) ----
    x_d = nc.dram_tensor("x", [T, DIM], f32, kind="ExternalInput")
    # lhsT for Q/K-ext matmul: ch-halves folded [128, 2(kk), 512(M)]
    wA_d = nc.dram_tensor("wA", [128, 2, 512], bf16, kind="ExternalInput")
    # rhs for V matmul: [128, 2(kk), 512(N)] col h*64+j = Wv rows
    wV_d = nc.dram_tensor("wV", [128, 2, DH], bf16, kind="ExternalInput")
    # proj lhsT, folded: [128, 4, 256]; contraction halves (see host prep)
    wP_d = nc.dram_tensor("wP", [128, 4, DIM], bf16, kind="ExternalInput")
    bP_d = nc.dram_tensor("bP", [128, 2], f32, kind="ExternalInput")
    ones_d = nc.dram_tensor("ones8", [H, T], bf16, kind="ExternalInput")
    # exp(bias) transposed: [h, m, n] viewed [H*8, 128, 1024]
    eb_d = nc.dram_tensor("ebT", [H * 8, 128, N], bf16, kind="ExternalInput")
    out_d = nc.dram_tensor("outT", [2, 128, T], f32, kind="ExternalOutput")

    NT = T // 128  # 32 token tiles

    with tile.TileContext(nc) as tc:
        with tc.tile_pool(name="const", bufs=1) as const_pool:
            ident = const_pool.tile([128, 128], bf16)
            make_identity(nc, ident)
            eps_t = const_pool.tile([128, 1], f32)
            nc.vector.memset(eps_t, EPS)
            wP_sb = const_pool.tile([128, 4, DIM], bf16)
            nc.sync.dma_start(out=wP_sb, in_=wP_d.ap())
            bP_sb = const_pool.tile([128, 2], f32)
            nc.sync.dma_start(out=bP_sb, in_=bP_d.ap())

            # Persistent activations
            qk_sb = const_pool.tile([128, 4, T], bf16)     # [Qext;Kext] rows folded
            v_sb = const_pool.tile([128, NT, 8, 65], bf16)  # v per tok-tile +ones col
            attn_pT = const_pool.tile([128, 4, T], bf16)   # normalized attn out^T

            # ---------- Phases A-C (xnT and matmul weights are scoped) -----
            acts_ctx = tc.tile_pool(name="acts", bufs=1)
            acts_pool = acts_ctx.__enter__()
            wA_sb = acts_pool.tile([128, 2, 512], bf16, tag="wA")
            nc.sync.dma_start(out=wA_sb, in_=wA_d.ap())
            wV_sb = acts_pool.tile([128, 2, DH], bf16, tag="wV")
            nc.sync.dma_start(out=wV_sb, in_=wV_d.ap())
            xnT = acts_pool.tile([128, 2, T], bf16, tag="xnT")
            # ---------- Phase A: LayerNorm (token layout) + PE transpose ----
            with tc.tile_pool(name="ln", bufs=3) as ln_pool, \
                 tc.tile_pool(name="lnp", bufs=4, space="PSUM") as lnp_pool:
                for t in range(NT):
                    x_t = ln_pool.tile([128, DIM], f32, tag="x")
                    nc.sync.dma_start(out=x_t, in_=x_d.ap()[t * 128:(t + 1) * 128, :])
                    stats = ln_pool.tile([128, 6], f32, tag="st")
                    nc.vector.bn_stats(out=stats, in_=x_t)
                    mv = ln_pool.tile([128, 2], f32, tag="mv")
                    nc.vector.bn_aggr(out=mv, in_=stats)
                    # rstd = 1/sqrt(var+eps)
                    std = ln_pool.tile([128, 1], f32, tag="sd")
                    nc.scalar.activation(out=std, in_=mv[:, 1:2], func=AF.Sqrt,
                                         bias=eps_t, scale=1.0)
                    nc.vector.reciprocal(out=std, in_=std)
                    xn_t = ln_pool.tile([128, DIM], bf16, tag="xn")
                    nc.vector.tensor_scalar(out=xn_t, in0=x_t,
                                            scalar1=mv[:, 0:1], scalar2=std,
                                            op0=ALU.subtract, op1=ALU.mult)
                    for kk in range(2):
                        ps_t = lnp_pool.tile([128, 128], bf16, tag="tp")
                        nc.tensor.transpose(ps_t, xn_t[:, kk * 128:(kk + 1) * 128],
                                            ident)
                        nc.vector.tensor_copy(
                            out=xnT[:, kk, t * 128:(t + 1) * 128], in_=ps_t)

            # ---------- Phase B: Q/K-ext matmul  out[512, T] ----------------
            with tc.tile_pool(name="qkp", bufs=4, space="PSUM") as qkp_pool:
                for j in range(T // 512):
                    for m in range(4):
                        ps = qkp_pool.tile([128, 512], f32, tag="qk")
                        for kk in range(2):
                            nc.tensor.matmul(
                                ps,
                                lhsT=wA_sb[:, kk, m * 128:(m + 1) * 128],
                                rhs=xnT[:, kk, j * 512:(j + 1) * 512],
                                start=(kk == 0), stop=(kk == 1))
                        nc.scalar.activation(
                            out=qk_sb[:, m, j * 512:(j + 1) * 512], in_=ps,
                            func=AF.Copy)
            # ones rows for the q-side bias-correction channel: row h*32+16
            for h in range(H):
                p = (h * 32 + 16) % 128
                blk = (h * 32) // 128
                nc.sync.dma_start(out=qk_sb[p:p + 1, blk, :],
                                  in_=ones_d.ap()[h:h + 1, :])

            # ---------- Phase C: V matmul  v_tok[T, 512] (+ones cols) -------
            with tc.tile_pool(name="vp", bufs=4, space="PSUM") as vp_pool:
                for t in range(NT):
                    ps = vp_pool.tile([128, 512], f32, tag="v")
                    for kk in range(2):
                        nc.tensor.matmul(
                            ps, lhsT=xnT[:, kk, t * 128:(t + 1) * 128],
                            rhs=wV_sb[:, kk, :],
                            start=(kk == 0), stop=(kk == 1))
                    # spread heads into 65-wide blocks (col 64 = ones)
                    nc.vector.tensor_copy(
                        out=v_sb[:, t, :, 0:64],
                        in_=ps.rearrange("p (h d) -> p h d", h=8))
                    nc.vector.memset(v_sb[:, t, :, 64:65], 1.0)

            acts_ctx.__exit__(None, None, None)

            # ---------- Phase D: attention, joint head pairs ----------------
            # Per (hp, b): interleaved score MMs for both heads (PE row-group
            # packing), exp on ACT, exp(bias) multiply on DVE per 4-mt half,
            # AV accumulation, then inline Z-reciprocal + PSUM-broadcast
            # normalization (no DRAM bounce, no tail).
            ones1 = const_pool.tile([33, 64], bf16)
            nc.vector.memset(ones1, 1.0)
            with tc.tile_pool(name="eb", bufs=1) as eb_pool, \
                 tc.tile_pool(name="at", bufs=5) as at_pool, \
                 tc.tile_pool(name="avs", bufs=3) as avs_pool, \
                 tc.tile_pool(name="zp", bufs=3) as z_pool, \
                 tc.tile_pool(name="sp", bufs=2, space="PSUM") as sp_pool, \
                 tc.tile_pool(name="avp", bufs=2, space="PSUM") as avp_pool:
                for hp in range(H // 2):
                    eb_sb = eb_pool.tile([128, 16, N], bf16, tag="eb")
                    nc.sync.dma_start(
                        out=eb_sb,
                        in_=eb_d.ap()[hp * 16:(hp + 1) * 16, :, :]
                        .rearrange("c p n -> p c n"))
                    for b in range(BL):
                        ps_avs = [avp_pool.tile([128, N], f32, tag="av", name=f"av{_i}")
                                  for _i in range(2)]
                        for half in range(2):
                            ats = [at_pool.tile([128, 4, N], bf16, tag="at", name=f"at{_i}")
                                   for _i in range(2)]
                            for mt4 in range(4):
                                mt = half * 4 + mt4
                                pss = [sp_pool.tile([128, N], f32, tag="s", name=f"s{_i}")
                                       for _i in range(2)]
                                for nc2 in range(2):
                                    for h2 in range(2):
                                        h = hp * 2 + h2
                                        base = 32 * (h % 4)
                                        blkq = h // 4
                                        nc.tensor.matmul(
                                            pss[h2][:, nc2 * 512:(nc2 + 1) * 512],
                                            lhsT=qk_sb[base:base + 32, 2 + blkq,
                                                       b * N + mt * 128:
                                                       b * N + (mt + 1) * 128],
                                            rhs=qk_sb[base:base + 32, blkq,
                                                      b * N + nc2 * 512:
                                                      b * N + (nc2 + 1) * 512],
                                            start=True, stop=True,
                                            tile_position=(base, 0))
                                for h2 in range(2):
                                    nc.scalar.activation(
                                        out=ats[h2][:, mt4, :], in_=pss[h2],
                                        func=AF.Exp)
                            for h2 in range(2):
                                nc.vector.tensor_mul(
                                    out=ats[h2].rearrange("p a n -> p (a n)"),
                                    in0=ats[h2].rearrange("p a n -> p (a n)"),
                                    in1=eb_sb[:, h2 * 8 + half * 4:
                                              h2 * 8 + (half + 1) * 4, :]
                                    .rearrange("p a n -> p (a n)"))
                            for h2 in range(2):
                                h = hp * 2 + h2
                                for mt4 in range(4):
                                    mt = half * 4 + mt4
                                    for nc2 in range(2):
                                        nc.tensor.matmul(
                                            ps_avs[h2][0:65,
                                                       nc2 * 512:(nc2 + 1) * 512],
                                            lhsT=v_sb[:, b * 8 + mt, h, :],
                                            rhs=ats[h2][:, mt4,
                                                        nc2 * 512:(nc2 + 1) * 512],
                                            start=(mt == 0), stop=(mt == 7))
                        # drain AV, collect Z, normalize the pair in place
                        zpair = z_pool.tile([33, N], bf16, tag="z")
                        for h2 in range(2):
                            h = hp * 2 + h2
                            av_sb = avs_pool.tile([65, N], bf16, tag="avs")
                            nc.vector.tensor_copy(out=av_sb,
                                                  in_=ps_avs[h2][0:65, :])
                            if h2 == 0:
                                nc.gpsimd.tensor_copy(
                                    out=attn_pT[0:64, hp, b * N:(b + 1) * N],
                                    in_=av_sb[0:64, :])
                            else:
                                nc.sync.dma_start(
                                    out=attn_pT[64:128, hp, b * N:(b + 1) * N],
                                    in_=av_sb[0:64, :])
                            nc.sync.dma_start(out=zpair[h2:h2 + 1, :],
                                              in_=av_sb[64:65, :])
                        with nc.allow_low_precision(reason="1/Z in bf16: 0.2% rel, fine for 2e-2 gate"):
                            nc.vector.reciprocal(out=zpair[0:2, :], in_=zpair[0:2, :])
                        # move h1's 1/Z row to a 32-aligned partition for the MM
                        nc.sync.dma_start(out=zpair[32:33, :], in_=zpair[1:2, :])
                        ps_rz = sp_pool.tile([128, N], f32, tag="s")
                        for h2 in range(2):
                            for nc2 in range(2):
                                nc.tensor.matmul(
                                    ps_rz[h2 * 64:(h2 + 1) * 64,
                                          nc2 * 512:(nc2 + 1) * 512],
                                    lhsT=ones1[32 * h2:32 * h2 + 1, :],
                                    rhs=zpair[32 * h2:32 * h2 + 1,
                                              nc2 * 512:(nc2 + 1) * 512],
                                    start=True, stop=True,
                                    tile_position=(32 * h2, 64 * h2))
                        nc.vector.tensor_mul(
                            out=attn_pT[:, hp, b * N:(b + 1) * N],
                            in0=attn_pT[:, hp, b * N:(b + 1) * N],
                            in1=ps_rz)

            # ---------- Phase E: output projection --------------------------
            with tc.tile_pool(name="po", bufs=3) as po_pool, \
                 tc.tile_pool(name="pp", bufs=4, space="PSUM") as pp_pool:
                for j in range(T // 512):
                    for mo in range(2):
                        ps = pp_pool.tile([128, 512], f32, tag="p")
                        for kk in range(4):
                            nc.tensor.matmul(
                                ps,
                                lhsT=wP_sb[:, kk, mo * 128:(mo + 1) * 128],
                                rhs=attn_pT[:, kk, j * 512:(j + 1) * 512],
                                start=(kk == 0), stop=(kk == 3))
                        o_sb = po_pool.tile([128, 512], f32, tag="o")
                        nc.vector.tensor_scalar(
                            out=o_sb, in0=ps,
                            scalar1=bP_sb[:, mo:mo + 1],
                            scalar2=None, op0=ALU.add)
                        nc.sync.dma_start(
                            out=out_d.ap()[mo, :, j * 512:(j + 1) * 512],
                            in_=o_sb)

    nc.compile()
    return nc


def _host_prep(gamma, beta, w_qkv, b_qkv, w_proj, b_proj, attn_biases,
               bias_idxs):
    """Fold biases/affines into weights; gather+exp the bias table."""
    w_eff = (w_qkv * gamma[None, :]).astype(np.float32)
    b_eff = (w_qkv @ beta + b_qkv).astype(np.float32)
    wq = np.zeros((H, KD, DIM), np.float32)
    wk = np.zeros((H, KD, DIM), np.float32)
    wv = np.zeros((H, D, DIM), np.float32)
    bq = np.zeros((H, KD), np.float32)
    bv = np.zeros((H, D), np.float32)
    for h in range(H):
        r0 = h * (2 * KD + D)
        wq[h] = w_eff[r0:r0 + KD]
        wk[h] = w_eff[r0 + KD:r0 + 2 * KD]
        wv[h] = w_eff[r0 + 2 * KD:r0 + 2 * KD + D]
        bq[h] = b_eff[r0:r0 + KD]
        bv[h] = b_eff[r0 + 2 * KD:r0 + 2 * KD + D]

    # wA: [256, 512] cols = Qext | Kext blocks of 32 per head
    wA = np.zeros((DIM, 512), np.float32)
    for h in range(H):
        wA[:, h * 32:h * 32 + KD] = (SCALE * wq[h]).T
        wA[:, 256 + h * 32:256 + h * 32 + KD] = wk[h].T
        # extra channel: scale * (bq_h @ Wk_h)
        wA[:, 256 + h * 32 + KD] = SCALE * (bq[h] @ wk[h])
    wV = np.zeros((DIM, DH), np.float32)
    for h in range(H):
        wV[:, h * D:(h + 1) * D] = wv[h].T
    wA = np.ascontiguousarray(wA.reshape(2, 128, 512).transpose(1, 0, 2))
    wV = np.ascontiguousarray(wV.reshape(2, 128, DH).transpose(1, 0, 2))

    # proj lhsT with the attn_pT folded layout: contraction row (h, d) lives at
    # partition d, free-block h  ->  wP[d, h, c] = w_proj[c, h*64+d]
    wP = np.zeros((128, 4, DIM), np.float32)
    wpr = w_proj.reshape(DIM, H, D)  # [c, h, d]
    for h in range(H):
        wP[(h % 2) * 64:(h % 2) * 64 + 64, h // 2, :] = wpr[:, h, :].T
    bP = np.ascontiguousarray((b_proj + w_proj @ bv.reshape(DH)).astype(np.float32).reshape(2, 128).T)

    ebT = np.exp(attn_biases.astype(np.float32))[:, bias_idxs.T]  # [H, m, n]
    ebT = np.ascontiguousarray(ebT.reshape(H * 8, 128, N)).astype(_BF16)
    ones8 = np.ones((H, T), _BF16)
    return (wA.astype(_BF16), wV.astype(_BF16), wP.astype(_BF16),
            bP.astype(np.float32), ebT, ones8)


def _register_ntff_hook():
    """The container's antenv stub lacks axon_hooks; synthesize it so
    run_bass_kernel_spmd(trace=True) can capture NTFF profiles."""
    import types
    if "antenv.axon_hooks" in sys.modules:
        return
    try:
        from trn_agent_boot.trn_boot import _ntff_profile_via_ctypes
        mod = types.ModuleType("antenv.axon_hooks")
        _state = {"hook": None}
        mod.set_axon_ntff_profile_hook = lambda h: _state.__setitem__("hook", h)
        mod.get_axon_ntff_profile_hook = lambda: _state["hook"]
        sys.modules["antenv.axon_hooks"] = mod
        mod.set_axon_ntff_profile_hook(
            _ntff_profile_via_ctypes("/opt/axon/libaxon_pjrt.so"))
    except Exception:
        pass


def kernel(x, gamma, beta, w_qkv, b_qkv, w_proj, b_proj, attn_biases,
           bias_idxs):
    from concourse.bass_utils import run_bass_kernel_spmd

    x = np.asarray(x, np.float32)
    gamma = np.asarray(gamma, np.float32)
    beta = np.asarray(beta, np.float32)
    w_qkv = np.asarray(w_qkv, np.float32)
    b_qkv = np.asarray(b_qkv, np.float32)
    w_proj = np.asarray(w_proj, np.float32)
    b_proj = np.asarray(b_proj, np.float32)
    attn_biases = np.asarray(attn_biases, np.float32)
    bias_idxs = np.asarray(bias_idxs, np.int32)

    wA, wV, wP, bP, ebT, ones8 = _host_prep(
        gamma, beta, w_qkv, b_qkv, w_proj, b_proj, attn_biases, bias_idxs)

    if "nc" not in _CACHE:
        _CACHE["nc"] = _build()
    nc = _CACHE["nc"]

    in_maps = []
    for c in range(NCORES):
        xs = np.ascontiguousarray(
            x[c * BL:(c + 1) * BL].reshape(T, DIM)).astype(np.float32)
        in_maps.append({
            "x": xs, "wA": wA, "wV": wV, "wP": wP, "bP": bP,
            "ones8": ones8, "ebT": ebT,
        })

    trace = bool(int(os.environ.get("BASS_TRACE_RUN", "0")))
    if trace:
        _register_ntff_hook()
    try:
        res = run_bass_kernel_spmd(nc, in_maps,
                                   core_ids=list(range(NCORES)), trace=trace)
    except Exception:
        if not trace:
            raise
        res = run_bass_kernel_spmd(nc, in_maps,
                                   core_ids=list(range(NCORES)), trace=False)
    _CACHE["last_result"] = res
    outs = []
    for c in range(NCORES):
        oT = res.results[c]["outT"]  # [2, 128, T] f32
        o = oT.reshape(DIM, T).T.reshape(BL, N, DIM)
        outs.append(o)
    return np.concatenate(outs, 0).astype(np.float32)


# revision 19
# speedup vs baseline: 1.3891x; 1.3891x over previous
"""Trainium2 Bass kernel for nn_Attention_22454089023887 (sparse_attention).

LayerNorm -> QKV -> 8-head attention with gathered rel-pos bias -> softmax -> proj.
Sharding: data-parallel over batch B=32 across 8 cores (4 batches/core), no
collectives.  The bias gather attn_biases[:, bias_idxs] has no efficient device
primitive (GPSIMD gather is ~100x too slow for 8M elements), so exp(bias) is
precomputed host-side and streamed as a bf16 input; the softmax applies it
multiplicatively: exp(s + b) = exp(s) * exp(b).

Device-side layout choices (see comments inline):
  - scores are computed transposed, ST[m, n], so the AV matmul can contract
    over m on partitions with no attention-matrix transpose;
  - softmax Z comes free from a ones-column appended to V (row 64 of AV psum);
  - all LN/QKV bias terms that are softmax-row-constants are dropped; the
    m-varying term scale*bq.k0[m] enters as an extra QKV output channel;
  - v-bias and beta/gamma fold into host-side weight preprocessing.
"""

import sys
import os

sys.path.insert(0, "/opt/trn_rl_repo")

import numpy as np
import ml_dtypes

B, N, DIM = 32, 1024, 256
H, KD, AR = 8, 16, 4
D = AR * KD  # 64
DH = D * H  # 512
SCALE = KD ** (-0.5)  # 0.25
EPS = 1e-5
NCORES = 8
BL = B // NCORES  # 4 batches per core
T = BL * N  # 4096 tokens per core
BH = BL * H  # 32 (b, h) pairs per core

_BF16 = ml_dtypes.bfloat16

_CACHE = {}


def _build():
    """Build the Bass graph once; returns (nc, names) for run_bass_kernel_spmd."""
    import concourse.bass as bass
    import concourse.tile as tile
    from concourse import bacc, mybir
    from concourse.masks import make_identity

    f32 = mybir.dt.float32
    bf16 = mybir.dt.bfloat16
    AF = mybir.ActivationFunctionType
    ALU = mybir.AluOpType

    nc = bacc.Bacc("TRN2", target_bir_lowering=False, debug=False,
                   num_devices=NCORES)

    # ---- DRAM parameters (per-core shards / replicated tables) ----
    x_d = nc.dram_tensor("x", [T, DIM], f32, kind="ExternalInput")
    # lhsT for Q/K-ext matmul: ch-halves folded [128, 2(kk), 512(M)]
    wA_d = nc.dram_tensor("wA", [128, 2, 512], bf16, kind="ExternalInput")
    # rhs for V matmul: [128, 2(kk), 512(N)] col h*64+j = Wv rows
    wV_d = nc.dram_tensor("wV", [128, 2, DH], bf16, kind="ExternalInput")
    # proj lhsT, folded: [128, 4, 256]; contraction halves (see host prep)
    wP_d = nc.dram_tensor("wP", [128, 4, DIM], bf16, kind="ExternalInput")
    bP_d = nc.dram_tensor("bP", [128, 2], f32, kind="ExternalInput")
    ones_d = nc.dram_tensor("ones8", [H, T], bf16, kind="ExternalInput")
    # exp(bias) transposed: [h, m, n] viewed [H*8, 128, 1024]
    eb_d = nc.dram_tensor("ebT", [H * 8, 128, N], bf16, kind="ExternalInput")
    out_d = nc.dram_tensor("outT", [2, 128, T], f32, kind="ExternalOutput")
    ap_d = nc.dram_tensor("attn_pT_dram", [128, 4, T], mybir.dt.bfloat16,
                          kind="Internal")

    NT = T // 128  # 32 token tiles

    with tile.TileContext(nc) as tc:
        with tc.tile_pool(name="const", bufs=1) as const_pool:
            ident = const_pool.tile([128, 128], bf16)
            make_identity(nc, ident)
            eps_t = const_pool.tile([128, 1], f32)
            nc.vector.memset(eps_t, EPS)
            wP_sb = const_pool.tile([128, 4, DIM], bf16)
            nc.sync.dma_start(out=wP_sb, in_=wP_d.ap())
            bP_sb = const_pool.tile([128, 2], f32)
            nc.sync.dma_start(out=bP_sb, in_=bP_d.ap())

            # Persistent activations
            qk_sb = const_pool.tile([128, 4, T], bf16)     # [Qext;Kext] rows folded
            v_sb = const_pool.tile([128, NT, 8, 65], bf16)  # v per tok-tile +ones col

            # ---------- Phases A-C (xnT and matmul weights are scoped) -----
            acts_ctx = tc.tile_pool(name="acts", bufs=1)
            acts_pool = acts_ctx.__enter__()
            wA_sb = acts_pool.tile([128, 2, 512], bf16, tag="wA")
            nc.sync.dma_start(out=wA_sb, in_=wA_d.ap())
            wV_sb = acts_pool.tile([128, 2, DH], bf16, tag="wV")
            nc.sync.dma_start(out=wV_sb, in_=wV_d.ap())
            xnT = acts_pool.tile([128, 2, T], bf16, tag="xnT")
            # ---------- Phase A: LayerNorm (token layout) + PE transpose ----
            with tc.tile_pool(name="ln", bufs=3) as ln_pool, \
                 tc.tile_pool(name="lnp", bufs=4, space="PSUM") as lnp_pool:
                for t in range(NT):
                    x_t = ln_pool.tile([128, DIM], f32, tag="x")
                    nc.sync.dma_start(out=x_t, in_=x_d.ap()[t * 128:(t + 1) * 128, :])
                    stats = ln_pool.tile([128, 6], f32, tag="st")
                    nc.vector.bn_stats(out=stats, in_=x_t)
                    mv = ln_pool.tile([128, 2], f32, tag="mv")
                    nc.vector.bn_aggr(out=mv, in_=stats)
                    # rstd = 1/sqrt(var+eps)
                    std = ln_pool.tile([128, 1], f32, tag="sd")
                    nc.scalar.activation(out=std, in_=mv[:, 1:2], func=AF.Sqrt,
                                         bias=eps_t, scale=1.0)
                    nc.vector.reciprocal(out=std, in_=std)
                    xn_t = ln_pool.tile([128, DIM], bf16, tag="xn")
                    nc.vector.tensor_scalar(out=xn_t, in0=x_t,
                                            scalar1=mv[:, 0:1], scalar2=std,
                                            op0=ALU.subtract, op1=ALU.mult)
                    for kk in range(2):
                        ps_t = lnp_pool.tile([128, 128], bf16, tag="tp")
                        nc.tensor.transpose(ps_t, xn_t[:, kk * 128:(kk + 1) * 128],
                                            ident)
                        nc.vector.tensor_copy(
                            out=xnT[:, kk, t * 128:(t + 1) * 128], in_=ps_t)

            # ---------- Phase B: Q/K-ext matmul  out[512, T] ----------------
            with tc.tile_pool(name="qkp", bufs=4, space="PSUM") as qkp_pool:
                for j in range(T // 512):
                    for m in range(4):
                        ps = qkp_pool.tile([128, 512], f32, tag="qk")
                        for kk in range(2):
                            nc.tensor.matmul(
                                ps,
                                lhsT=wA_sb[:, kk, m * 128:(m + 1) * 128],
                                rhs=xnT[:, kk, j * 512:(j + 1) * 512],
                                start=(kk == 0), stop=(kk == 1))
                        nc.scalar.activation(
                            out=qk_sb[:, m, j * 512:(j + 1) * 512], in_=ps,
                            func=AF.Copy)
            # ones rows for the q-side bias-correction channel: row h*32+16
            for h in range(H):
                p = (h * 32 + 16) % 128
                blk = (h * 32) // 128
                nc.sync.dma_start(out=qk_sb[p:p + 1, blk, :],
                                  in_=ones_d.ap()[h:h + 1, :])

            # ---------- Phase C: V matmul  v_tok[T, 512] (+ones cols) -------
            with tc.tile_pool(name="vp", bufs=4, space="PSUM") as vp_pool:
                for t in range(NT):
                    ps = vp_pool.tile([128, 512], f32, tag="v")
                    for kk in range(2):
                        nc.tensor.matmul(
                            ps, lhsT=xnT[:, kk, t * 128:(t + 1) * 128],
                            rhs=wV_sb[:, kk, :],
                            start=(kk == 0), stop=(kk == 1))
                    # spread heads into 65-wide blocks (col 64 = ones)
                    nc.vector.tensor_copy(
                        out=v_sb[:, t, :, 0:64],
                        in_=ps.rearrange("p (h d) -> p h d", h=8))
                    nc.vector.memset(v_sb[:, t, :, 64:65], 1.0)

            acts_ctx.__exit__(None, None, None)

            # ---------- Phase D: attention, joint head pairs ----------------
            # Per (hp, b): interleaved score MMs for both heads (PE row-group
            # packing), exp on ACT, exp(bias) multiply on DVE per 4-mt half,
            # AV accumulation, then inline Z handling: partition-spread
            # reciprocal, ones-matmul broadcast of 1/Z into PSUM, in-place
            # normalization of the AV drain, store to DRAM.
            ones1 = const_pool.tile([33, 64], bf16)
            nc.vector.memset(ones1, 1.0)
            with tc.tile_pool(name="eb", bufs=3) as eb_pool, \
                 tc.tile_pool(name="at", bufs=6) as at_pool, \
                 tc.tile_pool(name="avs", bufs=3) as avs_pool, \
                 tc.tile_pool(name="zp", bufs=3) as z_pool, \
                 tc.tile_pool(name="sp", bufs=2, space="PSUM") as sp_pool, \
                 tc.tile_pool(name="avp", bufs=2, space="PSUM") as avp_pool:
                for hp in range(H // 2):
                    ebs = [eb_pool.tile([128, 8, N], bf16, tag="eb",
                                        name=f"eb{hp}_{_i}") for _i in range(2)]
                    for h2 in range(2):
                        nc.sync.dma_start(
                            out=ebs[h2],
                            in_=eb_d.ap()[hp * 16 + h2 * 8:
                                          hp * 16 + (h2 + 1) * 8, :, :]
                            .rearrange("c p n -> p c n"))
                    for b in range(BL):
                        ps_avs = [avp_pool.tile([128, N], f32, tag="av",
                                                name=f"av{_i}")
                                  for _i in range(2)]
                        for half in range(2):
                            ats = [at_pool.tile([128, 4, N], bf16, tag="at",
                                                name=f"at{_i}")
                                   for _i in range(2)]
                            for mt4 in range(4):
                                mt = half * 4 + mt4
                                pss = [sp_pool.tile([128, N], f32, tag="s",
                                                    name=f"s{_i}")
                                       for _i in range(2)]
                                for nc2 in range(2):
                                    for h2 in range(2):
                                        h = hp * 2 + h2
                                        base = 32 * (h % 4)
                                        blkq = h // 4
                                        nc.tensor.matmul(
                                            pss[h2][:, nc2 * 512:(nc2 + 1) * 512],
                                            lhsT=qk_sb[base:base + 32, 2 + blkq,
                                                       b * N + mt * 128:
                                                       b * N + (mt + 1) * 128],
                                            rhs=qk_sb[base:base + 32, blkq,
                                                      b * N + nc2 * 512:
                                                      b * N + (nc2 + 1) * 512],
                                            start=True, stop=True,
                                            tile_position=(base, 0))
                                for h2 in range(2):
                                    nc.scalar.activation(
                                        out=ats[h2][:, mt4, :], in_=pss[h2],
                                        func=AF.Exp)
                            for h2 in range(2):
                                nc.vector.tensor_mul(
                                    out=ats[h2].rearrange("p a n -> p (a n)"),
                                    in0=ats[h2].rearrange("p a n -> p (a n)"),
                                    in1=ebs[h2][:, half * 4:(half + 1) * 4, :]
                                    .rearrange("p a n -> p (a n)"))
                            for h2 in range(2):
                                h = hp * 2 + h2
                                for mt4 in range(4):
                                    mt = half * 4 + mt4
                                    for nc2 in range(2):
                                        nc.tensor.matmul(
                                            ps_avs[h2][0:65,
                                                       nc2 * 512:(nc2 + 1) * 512],
                                            lhsT=v_sb[:, b * 8 + mt, h, :],
                                            rhs=ats[h2][:, mt4,
                                                        nc2 * 512:(nc2 + 1) * 512],
                                            start=(mt == 0), stop=(mt == 7))
                        # drain AV, spread-reciprocal Z, normalize, store
                        av_sbs = []
                        zsp = z_pool.tile([128, 16], bf16, tag="zsp")
                        zrow = z_pool.tile([33, N], bf16, tag="zrow")
                        for h2 in range(2):
                            av_sb = avs_pool.tile([65, N], bf16, tag="avs",
                                                  name=f"avsb{h2}")
                            av_sbs.append(av_sb)
                            nc.vector.tensor_copy(out=av_sb,
                                                  in_=ps_avs[h2][0:65, :])
                            nc.sync.dma_start(
                                out=zsp[:, h2 * 8:(h2 + 1) * 8],
                                in_=av_sb[64:65, :])
                        with nc.allow_low_precision(reason="1/Z bf16 ok"):
                            nc.vector.reciprocal(out=zsp, in_=zsp)
                        for h2 in range(2):
                            nc.sync.dma_start(
                                out=zrow[32 * h2:32 * h2 + 1, :],
                                in_=zsp[:, h2 * 8:(h2 + 1) * 8])
                        for h2 in range(2):
                            h = hp * 2 + h2
                            ps_rz = avp_pool.tile([64, N], f32, tag="av",
                                                  name=f"rz{h2}")
                            for nc2 in range(2):
                                nc.tensor.matmul(
                                    ps_rz[:, nc2 * 512:(nc2 + 1) * 512],
                                    lhsT=ones1[32 * h2:32 * h2 + 1, :],
                                    rhs=zrow[32 * h2:32 * h2 + 1,
                                             nc2 * 512:(nc2 + 1) * 512],
                                    start=True, stop=True,
                                    tile_position=(32 * h2, 0))
                            nc.vector.tensor_mul(
                                out=av_sbs[h2][0:64, :],
                                in0=av_sbs[h2][0:64, :],
                                in1=ps_rz)
                            nc.sync.dma_start(
                                out=ap_d.ap()[(h % 2) * 64:(h % 2) * 64 + 64,
                                              h // 2, b * N:(b + 1) * N],
                                in_=av_sbs[h2][0:64, :])

            # ---------- Phase E: output projection --------------------------
            with tc.tile_pool(name="po", bufs=3) as po_pool, \
                 tc.tile_pool(name="pp", bufs=4, space="PSUM") as pp_pool:
                for j in range(T // 512):
                    apt = po_pool.tile([128, 4, 512], bf16, tag="apt")
                    nc.sync.dma_start(out=apt,
                                      in_=ap_d.ap()[:, :, j * 512:(j + 1) * 512])
                    for mo in range(2):
                        ps = pp_pool.tile([128, 512], f32, tag="p")
                        for kk in range(4):
                            nc.tensor.matmul(
                                ps,
                                lhsT=wP_sb[:, kk, mo * 128:(mo + 1) * 128],
                                rhs=apt[:, kk, :],
                                start=(kk == 0), stop=(kk == 3))
                        o_sb = po_pool.tile([128, 512], f32, tag="o")
                        nc.vector.tensor_scalar(
                            out=o_sb, in0=ps,
                            scalar1=bP_sb[:, mo:mo + 1],
                            scalar2=None, op0=ALU.add)
                        nc.sync.dma_start(
                            out=out_d.ap()[mo, :, j * 512:(j + 1) * 512],
                            in_=o_sb)

    nc.compile()
    return nc


def _host_prep(gamma, beta, w_qkv, b_qkv, w_proj, b_proj, attn_biases,
               bias_idxs):
    """Fold biases/affines into weights; gather+exp the bias table."""
    w_eff = (w_qkv * gamma[None, :]).astype(np.float32)
    b_eff = (w_qkv @ beta + b_qkv).astype(np.float32)
    wq = np.zeros((H, KD, DIM), np.float32)
    wk = np.zeros((H, KD, DIM), np.float32)
    wv = np.zeros((H, D, DIM), np.float32)
    bq = np.zeros((H, KD), np.float32)
    bv = np.zeros((H, D), np.float32)
    for h in range(H):
        r0 = h * (2 * KD + D)
        wq[h] = w_eff[r0:r0 + KD]
        wk[h] = w_eff[r0 + KD:r0 + 2 * KD]
        wv[h] = w_eff[r0 + 2 * KD:r0 + 2 * KD + D]
        bq[h] = b_eff[r0:r0 + KD]
        bv[h] = b_eff[r0 + 2 * KD:r0 + 2 * KD + D]

    # wA: [256, 512] cols = Qext | Kext blocks of 32 per head
    wA = np.zeros((DIM, 512), np.float32)
    for h in range(H):
        wA[:, h * 32:h * 32 + KD] = (SCALE * wq[h]).T
        wA[:, 256 + h * 32:256 + h * 32 + KD] = wk[h].T
        # extra channel: scale * (bq_h @ Wk_h)
        wA[:, 256 + h * 32 + KD] = SCALE * (bq[h] @ wk[h])
    wV = np.zeros((DIM, DH), np.float32)
    for h in range(H):
        wV[:, h * D:(h + 1) * D] = wv[h].T
    wA = np.ascontiguousarray(wA.reshape(2, 128, 512).transpose(1, 0, 2))
    wV = np.ascontiguousarray(wV.reshape(2, 128, DH).transpose(1, 0, 2))

    # proj lhsT with the attn_pT folded layout: contraction row (h, d) lives at
    # partition d, free-block h  ->  wP[d, h, c] = w_proj[c, h*64+d]
    wP = np.zeros((128, 4, DIM), np.float32)
    wpr = w_proj.reshape(DIM, H, D)  # [c, h, d]
    for h in range(H):
        wP[(h % 2) * 64:(h % 2) * 64 + 64, h // 2, :] = wpr[:, h, :].T
    bP = np.ascontiguousarray((b_proj + w_proj @ bv.reshape(DH)).astype(np.float32).reshape(2, 128).T)

    ebT = np.exp(attn_biases.astype(np.float32))[:, bias_idxs.T]  # [H, m, n]
    ebT = np.ascontiguousarray(ebT.reshape(H * 8, 128, N)).astype(_BF16)
    ones8 = np.ones((H, T), _BF16)
    return (wA.astype(_BF16), wV.astype(_BF16), wP.astype(_BF16),
            bP.astype(np.float32), ebT, ones8)


def _register_ntff_hook():
    """The container's antenv stub lacks axon_hooks; synthesize it so
    run_bass_kernel_spmd(trace=True) can capture NTFF profiles."""
    import types
    if "antenv.axon_hooks" in sys.modules:
        return
    try:
        from trn_agent_boot.trn_boot import _ntff_profile_via_ctypes
        mod = types.ModuleType("antenv.axon_hooks")
        _state = {"hook": None}
        mod.set_axon_ntff_profile_hook = lambda h: _state.__setitem__("hook", h)
        mod.get_axon_ntff_profile_hook = lambda: _state["hook"]
        sys.modules["antenv.axon_hooks"] = mod
        mod.set_axon_ntff_profile_hook(
            _ntff_profile_via_ctypes("/opt/axon/libaxon_pjrt.so"))
    except Exception:
        pass


def kernel(x, gamma, beta, w_qkv, b_qkv, w_proj, b_proj, attn_biases,
           bias_idxs):
    from concourse.bass_utils import run_bass_kernel_spmd

    x = np.asarray(x, np.float32)
    gamma = np.asarray(gamma, np.float32)
    beta = np.asarray(beta, np.float32)
    w_qkv = np.asarray(w_qkv, np.float32)
    b_qkv = np.asarray(b_qkv, np.float32)
    w_proj = np.asarray(w_proj, np.float32)
    b_proj = np.asarray(b_proj, np.float32)
    attn_biases = np.asarray(attn_biases, np.float32)
    bias_idxs = np.asarray(bias_idxs, np.int32)

    wA, wV, wP, bP, ebT, ones8 = _host_prep(
        gamma, beta, w_qkv, b_qkv, w_proj, b_proj, attn_biases, bias_idxs)

    if "nc" not in _CACHE:
        _CACHE["nc"] = _build()
    nc = _CACHE["nc"]

    in_maps = []
    for c in range(NCORES):
        xs = np.ascontiguousarray(
            x[c * BL:(c + 1) * BL].reshape(T, DIM)).astype(np.float32)
        in_maps.append({
            "x": xs, "wA": wA, "wV": wV, "wP": wP, "bP": bP,
            "ones8": ones8, "ebT": ebT,
        })

    trace = bool(int(os.environ.get("BASS_TRACE_RUN", "0")))
    if trace:
        _register_ntff_hook()
    try:
        res = run_bass_kernel_spmd(nc, in_maps,
                                   core_ids=list(range(NCORES)), trace=trace)
    except Exception:
        if not trace:
            raise
        res = run_bass_kernel_spmd(nc, in_maps,
                                   core_ids=list(range(NCORES)), trace=False)
    _CACHE["last_result"] = res
    outs = []
    for c in range(NCORES):
        oT = res.results[c]["outT"]  # [2, 128, T] f32
        o = oT.reshape(DIM, T).T.reshape(BL, N, DIM)
        outs.append(o)
    return np.concatenate(outs, 0).astype(np.float32)


# revision 21
# speedup vs baseline: 1.4354x; 1.0333x over previous
"""Trainium2 Bass kernel for nn_Attention_22454089023887 (sparse_attention).

LayerNorm -> QKV -> 8-head attention with gathered rel-pos bias -> softmax -> proj.
Sharding: data-parallel over batch B=32 across 8 cores (4 batches/core), no
collectives.  The bias gather attn_biases[:, bias_idxs] has no efficient device
primitive (GPSIMD gather is ~100x too slow for 8M elements), so exp(bias) is
precomputed host-side and streamed as a bf16 input; the softmax applies it
multiplicatively: exp(s + b) = exp(s) * exp(b).

Device-side layout choices (see comments inline):
  - scores are computed transposed, ST[m, n], so the AV matmul can contract
    over m on partitions with no attention-matrix transpose;
  - softmax Z comes free from a ones-column appended to V (row 64 of AV psum);
  - all LN/QKV bias terms that are softmax-row-constants are dropped; the
    m-varying term scale*bq.k0[m] enters as an extra QKV output channel;
  - v-bias and beta/gamma fold into host-side weight preprocessing.
"""

import sys
import os

sys.path.insert(0, "/opt/trn_rl_repo")

import numpy as np
import ml_dtypes

B, N, DIM = 32, 1024, 256
H, KD, AR = 8, 16, 4
D = AR * KD  # 64
DH = D * H  # 512
SCALE = KD ** (-0.5)  # 0.25
EPS = 1e-5
NCORES = 8
BL = B // NCORES  # 4 batches per core
T = BL * N  # 4096 tokens per core
BH = BL * H  # 32 (b, h) pairs per core

_BF16 = ml_dtypes.bfloat16

_CACHE = {}


def _build():
    """Build the Bass graph once; returns (nc, names) for run_bass_kernel_spmd."""
    import concourse.bass as bass
    import concourse.tile as tile
    from concourse import bacc, mybir
    from concourse.masks import make_identity

    f32 = mybir.dt.float32
    bf16 = mybir.dt.bfloat16
    AF = mybir.ActivationFunctionType
    ALU = mybir.AluOpType

    nc = bacc.Bacc("TRN2", target_bir_lowering=False, debug=False,
                   num_devices=NCORES)

    # ---- DRAM parameters (per-core shards / replicated tables) ----
    x_d = nc.dram_tensor("x", [T, DIM], f32, kind="ExternalInput")
    # lhsT for Q/K-ext matmul: ch-halves folded [128, 2(kk), 512(M)]
    wA_d = nc.dram_tensor("wA", [128, 2, 512], bf16, kind="ExternalInput")
    # rhs for V matmul: [128, 2(kk), 512(N)] col h*64+j = Wv rows
    wV_d = nc.dram_tensor("wV", [128, 2, DH], bf16, kind="ExternalInput")
    # proj lhsT, folded: [128, 4, 256]; contraction halves (see host prep)
    wP_d = nc.dram_tensor("wP", [128, 4, DIM], bf16, kind="ExternalInput")
    bP_d = nc.dram_tensor("bP", [128, 2], f32, kind="ExternalInput")
    ones_d = nc.dram_tensor("ones8", [H, T], bf16, kind="ExternalInput")
    # exp(bias) transposed: [h, m, n] viewed [H*8, 128, 1024]
    eb_d = nc.dram_tensor("ebT", [H * 8, 128, N], bf16, kind="ExternalInput")
    out_d = nc.dram_tensor("outT", [2, 128, T], f32, kind="ExternalOutput")
    ap_d = nc.dram_tensor("attn_pT_dram", [128, 4, T], mybir.dt.bfloat16,
                          kind="Internal")
    rz_d = nc.dram_tensor("rz_dram", [BH, N], mybir.dt.bfloat16,
                          kind="Internal")

    NT = T // 128  # 32 token tiles

    with tile.TileContext(nc) as tc:
        with tc.tile_pool(name="const", bufs=1) as const_pool:
            ident = const_pool.tile([128, 128], bf16)
            make_identity(nc, ident)
            eps_t = const_pool.tile([128, 1], f32)
            nc.vector.memset(eps_t, EPS)
            wP_sb = const_pool.tile([128, 4, DIM], bf16)
            nc.sync.dma_start(out=wP_sb, in_=wP_d.ap())
            bP_sb = const_pool.tile([128, 2], f32)
            nc.sync.dma_start(out=bP_sb, in_=bP_d.ap())

            # Persistent activations
            qk_sb = const_pool.tile([128, 4, T], bf16)     # [Qext;Kext] rows folded
            v_sb = const_pool.tile([128, NT, 8, 65], bf16)  # v per tok-tile +ones col

            # ---------- Phases A-C (xnT and matmul weights are scoped) -----
            acts_ctx = tc.tile_pool(name="acts", bufs=1)
            acts_pool = acts_ctx.__enter__()
            wA_sb = acts_pool.tile([128, 2, 512], bf16, tag="wA")
            nc.sync.dma_start(out=wA_sb, in_=wA_d.ap())
            wV_sb = acts_pool.tile([128, 2, DH], bf16, tag="wV")
            nc.sync.dma_start(out=wV_sb, in_=wV_d.ap())
            xnT = acts_pool.tile([128, 2, T], bf16, tag="xnT")
            # ---------- Phase A: LayerNorm (token layout) + PE transpose ----
            with tc.tile_pool(name="ln", bufs=3) as ln_pool, \
                 tc.tile_pool(name="lnp", bufs=4, space="PSUM") as lnp_pool:
                for t in range(NT):
                    x_t = ln_pool.tile([128, DIM], f32, tag="x")
                    nc.sync.dma_start(out=x_t, in_=x_d.ap()[t * 128:(t + 1) * 128, :])
                    stats = ln_pool.tile([128, 6], f32, tag="st")
                    nc.vector.bn_stats(out=stats, in_=x_t)
                    mv = ln_pool.tile([128, 2], f32, tag="mv")
                    nc.vector.bn_aggr(out=mv, in_=stats)
                    # rstd = 1/sqrt(var+eps)
                    std = ln_pool.tile([128, 1], f32, tag="sd")
                    nc.scalar.activation(out=std, in_=mv[:, 1:2], func=AF.Sqrt,
                                         bias=eps_t, scale=1.0)
                    nc.vector.reciprocal(out=std, in_=std)
                    xn_t = ln_pool.tile([128, DIM], bf16, tag="xn")
                    nc.vector.tensor_scalar(out=xn_t, in0=x_t,
                                            scalar1=mv[:, 0:1], scalar2=std,
                                            op0=ALU.subtract, op1=ALU.mult)
                    for kk in range(2):
                        ps_t = lnp_pool.tile([128, 128], bf16, tag="tp")
                        nc.tensor.transpose(ps_t, xn_t[:, kk * 128:(kk + 1) * 128],
                                            ident)
                        nc.vector.tensor_copy(
                            out=xnT[:, kk, t * 128:(t + 1) * 128], in_=ps_t)

            # ---------- Phase B: Q/K-ext matmul  out[512, T] ----------------
            with tc.tile_pool(name="qkp", bufs=4, space="PSUM") as qkp_pool:
                for j in range(T // 512):
                    for m in range(4):
                        ps = qkp_pool.tile([128, 512], f32, tag="qk")
                        for kk in range(2):
                            nc.tensor.matmul(
                                ps,
                                lhsT=wA_sb[:, kk, m * 128:(m + 1) * 128],
                                rhs=xnT[:, kk, j * 512:(j + 1) * 512],
                                start=(kk == 0), stop=(kk == 1))
                        nc.scalar.activation(
                            out=qk_sb[:, m, j * 512:(j + 1) * 512], in_=ps,
                            func=AF.Copy)
            # ones rows for the q-side bias-correction channel: row h*32+16
            for h in range(H):
                p = (h * 32 + 16) % 128
                blk = (h * 32) // 128
                nc.sync.dma_start(out=qk_sb[p:p + 1, blk, :],
                                  in_=ones_d.ap()[h:h + 1, :])

            # ---------- Phase C: V matmul  v_tok[T, 512] (+ones cols) -------
            with tc.tile_pool(name="vp", bufs=4, space="PSUM") as vp_pool:
                for t in range(NT):
                    ps = vp_pool.tile([128, 512], f32, tag="v")
                    for kk in range(2):
                        nc.tensor.matmul(
                            ps, lhsT=xnT[:, kk, t * 128:(t + 1) * 128],
                            rhs=wV_sb[:, kk, :],
                            start=(kk == 0), stop=(kk == 1))
                    # spread heads into 65-wide blocks (col 64 = ones)
                    nc.vector.tensor_copy(
                        out=v_sb[:, t, :, 0:64],
                        in_=ps.rearrange("p (h d) -> p h d", h=8))
                    nc.vector.memset(v_sb[:, t, :, 64:65], 1.0)

            acts_ctx.__exit__(None, None, None)

            # ---------- Phase D: attention, joint head pairs ----------------
            # Scores at [128,1024] granularity with 3-deep PSUM pipelining;
            # AV runs nc2-sequential so its accumulators only hold 2 banks.
            # Z: ones-column of V -> row 64 of AV psum -> partition-spread
            # reciprocal -> DRAM bounce -> gpsimd replication DMA -> in-place
            # normalize of the drained AV tile -> store to DRAM.
            with tc.tile_pool(name="eb", bufs=3) as eb_pool, \
                 tc.tile_pool(name="at", bufs=6) as at_pool, \
                 tc.tile_pool(name="avs", bufs=5) as avs_pool, \
                 tc.tile_pool(name="zp", bufs=3) as z_pool, \
                 tc.tile_pool(name="rzp", bufs=3) as rz_pool, \
                 tc.tile_pool(name="sp", bufs=3, space="PSUM") as sp_pool, \
                 tc.tile_pool(name="avp", bufs=2, space="PSUM") as avp_pool:
                for hp in range(H // 2):
                    ebs = [eb_pool.tile([128, 8, N], bf16, tag="eb",
                                        name=f"eb{hp}_{_i}") for _i in range(2)]
                    for h2 in range(2):
                        nc.sync.dma_start(
                            out=ebs[h2],
                            in_=eb_d.ap()[hp * 16 + h2 * 8:
                                          hp * 16 + (h2 + 1) * 8, :, :]
                            .rearrange("c p n -> p c n"))
                    for b in range(BL):
                        all_ats = []
                        for half in range(2):
                            ats = [at_pool.tile([128, 4, N], bf16, tag="at",
                                                name=f"at{_i}")
                                   for _i in range(2)]
                            all_ats.append(ats)
                            for mt4 in range(4):
                                mt = half * 4 + mt4
                                pss = [sp_pool.tile([128, N], f32, tag="s",
                                                    name=f"s{_i}")
                                       for _i in range(2)]
                                for h2 in range(2):
                                    h = hp * 2 + h2
                                    base = 32 * (h % 4)
                                    blkq = h // 4
                                    for nc2 in range(2):
                                        nc.tensor.matmul(
                                            pss[h2][:, nc2 * 512:(nc2 + 1) * 512],
                                            lhsT=qk_sb[base:base + 32, 2 + blkq,
                                                       b * N + mt * 128:
                                                       b * N + (mt + 1) * 128],
                                            rhs=qk_sb[base:base + 32, blkq,
                                                      b * N + nc2 * 512:
                                                      b * N + (nc2 + 1) * 512],
                                            start=True, stop=True,
                                            tile_position=(base, 0))
                                for h2 in range(2):
                                    nc.scalar.activation(
                                        out=ats[h2][:, mt4, :], in_=pss[h2],
                                        func=AF.Exp)
                            for h2 in range(2):
                                nc.vector.tensor_mul(
                                    out=ats[h2].rearrange("p a n -> p (a n)"),
                                    in0=ats[h2].rearrange("p a n -> p (a n)"),
                                    in1=ebs[h2][:, half * 4:(half + 1) * 4, :]
                                    .rearrange("p a n -> p (a n)"))
                        # AV: nc2-sequential to keep psum to 2 banks
                        zsp = z_pool.tile([128, 16], bf16, tag="zsp")
                        av_sbs = {}
                        for nc2 in range(2):
                            ps_avs = [avp_pool.tile([65, 512], f32, tag="av",
                                                    name=f"av{_i}")
                                      for _i in range(2)]
                            for h2 in range(2):
                                h = hp * 2 + h2
                                for mt in range(8):
                                    nc.tensor.matmul(
                                        ps_avs[h2],
                                        lhsT=v_sb[:, b * 8 + mt, h, :],
                                        rhs=all_ats[mt // 4][h2][:, mt % 4,
                                                                 nc2 * 512:
                                                                 (nc2 + 1) * 512],
                                        start=(mt == 0), stop=(mt == 7))
                            for h2 in range(2):
                                av_sb = avs_pool.tile([65, 512], bf16, tag="avs",
                                                      name=f"avsb{h2}")
                                av_sbs[(h2, nc2)] = av_sb
                                nc.any.tensor_copy(out=av_sb, in_=ps_avs[h2])
                                nc.sync.dma_start(
                                    out=zsp[nc2 * 64:(nc2 + 1) * 64,
                                            h2 * 8:(h2 + 1) * 8],
                                    in_=av_sb[64:65, :])
                        with nc.allow_low_precision(reason="1/Z bf16 ok"):
                            nc.vector.reciprocal(out=zsp, in_=zsp)
                        for h2 in range(2):
                            h = hp * 2 + h2
                            nc.sync.dma_start(
                                out=rz_d.ap()[b * H + h:b * H + h + 1, :],
                                in_=zsp[:, h2 * 8:(h2 + 1) * 8])
                        for h2 in range(2):
                            h = hp * 2 + h2
                            rz = rz_pool.tile([64, N], bf16, tag="rz")
                            src = rz_d.ap()[b * H + h:b * H + h + 1, :]
                            nc.gpsimd.dma_start(
                                out=rz,
                                in_=bass.AP(tensor=src.tensor, offset=src.offset,
                                            ap=[[0, 64]] + src.ap[1:]))
                            for nc2 in range(2):
                                nc.vector.tensor_mul(
                                    out=av_sbs[(h2, nc2)][0:64, :],
                                    in0=av_sbs[(h2, nc2)][0:64, :],
                                    in1=rz[:, nc2 * 512:(nc2 + 1) * 512])
                                nc.sync.dma_start(
                                    out=ap_d.ap()[(h % 2) * 64:(h % 2) * 64 + 64,
                                                  h // 2,
                                                  b * N + nc2 * 512:
                                                  b * N + (nc2 + 1) * 512],
                                    in_=av_sbs[(h2, nc2)][0:64, :])

            # ---------- Phase E: output projection --------------------------
            with tc.tile_pool(name="po", bufs=3) as po_pool, \
                 tc.tile_pool(name="pp", bufs=4, space="PSUM") as pp_pool:
                for j in range(T // 512):
                    apt = po_pool.tile([128, 4, 512], bf16, tag="apt")
                    nc.sync.dma_start(out=apt,
                                      in_=ap_d.ap()[:, :, j * 512:(j + 1) * 512])
                    for mo in range(2):
                        ps = pp_pool.tile([128, 512], f32, tag="p")
                        for kk in range(4):
                            nc.tensor.matmul(
                                ps,
                                lhsT=wP_sb[:, kk, mo * 128:(mo + 1) * 128],
                                rhs=apt[:, kk, :],
                                start=(kk == 0), stop=(kk == 3))
                        o_sb = po_pool.tile([128, 512], f32, tag="o")
                        nc.vector.tensor_scalar(
                            out=o_sb, in0=ps,
                            scalar1=bP_sb[:, mo:mo + 1],
                            scalar2=None, op0=ALU.add)
                        nc.sync.dma_start(
                            out=out_d.ap()[mo, :, j * 512:(j + 1) * 512],
                            in_=o_sb)

    nc.compile()
    return nc


def _host_prep(gamma, beta, w_qkv, b_qkv, w_proj, b_proj, attn_biases,
               bias_idxs):
    """Fold biases/affines into weights; gather+exp the bias table."""
    w_eff = (w_qkv * gamma[None, :]).astype(np.float32)
    b_eff = (w_qkv @ beta + b_qkv).astype(np.float32)
    wq = np.zeros((H, KD, DIM), np.float32)
    wk = np.zeros((H, KD, DIM), np.float32)
    wv = np.zeros((H, D, DIM), np.float32)
    bq = np.zeros((H, KD), np.float32)
    bv = np.zeros((H, D), np.float32)
    for h in range(H):
        r0 = h * (2 * KD + D)
        wq[h] = w_eff[r0:r0 + KD]
        wk[h] = w_eff[r0 + KD:r0 + 2 * KD]
        wv[h] = w_eff[r0 + 2 * KD:r0 + 2 * KD + D]
        bq[h] = b_eff[r0:r0 + KD]
        bv[h] = b_eff[r0 + 2 * KD:r0 + 2 * KD + D]

    # wA: [256, 512] cols = Qext | Kext blocks of 32 per head
    wA = np.zeros((DIM, 512), np.float32)
    for h in range(H):
        wA[:, h * 32:h * 32 + KD] = (SCALE * wq[h]).T
        wA[:, 256 + h * 32:256 + h * 32 + KD] = wk[h].T
        # extra channel: scale * (bq_h @ Wk_h)
        wA[:, 256 + h * 32 + KD] = SCALE * (bq[h] @ wk[h])
    wV = np.zeros((DIM, DH), np.float32)
    for h in range(H):
        wV[:, h * D:(h + 1) * D] = wv[h].T
    wA = np.ascontiguousarray(wA.reshape(2, 128, 512).transpose(1, 0, 2))
    wV = np.ascontiguousarray(wV.reshape(2, 128, DH).transpose(1, 0, 2))

    # proj lhsT with the attn_pT folded layout: contraction row (h, d) lives at
    # partition d, free-block h  ->  wP[d, h, c] = w_proj[c, h*64+d]
    wP = np.zeros((128, 4, DIM), np.float32)
    wpr = w_proj.reshape(DIM, H, D)  # [c, h, d]
    for h in range(H):
        wP[(h % 2) * 64:(h % 2) * 64 + 64, h // 2, :] = wpr[:, h, :].T
    bP = np.ascontiguousarray((b_proj + w_proj @ bv.reshape(DH)).astype(np.float32).reshape(2, 128).T)

    ebT = np.exp(attn_biases.astype(np.float32))[:, bias_idxs.T]  # [H, m, n]
    ebT = np.ascontiguousarray(ebT.reshape(H * 8, 128, N)).astype(_BF16)
    ones8 = np.ones((H, T), _BF16)
    return (wA.astype(_BF16), wV.astype(_BF16), wP.astype(_BF16),
            bP.astype(np.float32), ebT, ones8)


def _register_ntff_hook():
    """The container's antenv stub lacks axon_hooks; synthesize it so
    run_bass_kernel_spmd(trace=True) can capture NTFF profiles."""
    import types
    if "antenv.axon_hooks" in sys.modules:
        return
    try:
        from trn_agent_boot.trn_boot import _ntff_profile_via_ctypes
        mod = types.ModuleType("antenv.axon_hooks")
        _state = {"hook": None}
        mod.set_axon_ntff_profile_hook = lambda h: _state.__setitem__("hook", h)
        mod.get_axon_ntff_profile_hook = lambda: _state["hook"]
        sys.modules["antenv.axon_hooks"] = mod
        mod.set_axon_ntff_profile_hook(
            _ntff_profile_via_ctypes("/opt/axon/libaxon_pjrt.so"))
    except Exception:
        pass


def kernel(x, gamma, beta, w_qkv, b_qkv, w_proj, b_proj, attn_biases,
           bias_idxs):
    from concourse.bass_utils import run_bass_kernel_spmd

    x = np.asarray(x, np.float32)
    gamma = np.asarray(gamma, np.float32)
    beta = np.asarray(beta, np.float32)
    w_qkv = np.asarray(w_qkv, np.float32)
    b_qkv = np.asarray(b_qkv, np.float32)
    w_proj = np.asarray(w_proj, np.float32)
    b_proj = np.asarray(b_proj, np.float32)
    attn_biases = np.asarray(attn_biases, np.float32)
    bias_idxs = np.asarray(bias_idxs, np.int32)

    wA, wV, wP, bP, ebT, ones8 = _host_prep(
        gamma, beta, w_qkv, b_qkv, w_proj, b_proj, attn_biases, bias_idxs)

    if "nc" not in _CACHE:
        _CACHE["nc"] = _build()
    nc = _CACHE["nc"]

    in_maps = []
    for c in range(NCORES):
        xs = np.ascontiguousarray(
            x[c * BL:(c + 1) * BL].reshape(T, DIM)).astype(np.float32)
        in_maps.append({
            "x": xs, "wA": wA, "wV": wV, "wP": wP, "bP": bP,
            "ones8": ones8, "ebT": ebT,
        })

    trace = bool(int(os.environ.get("BASS_TRACE_RUN", "0")))
    if trace:
        _register_ntff_hook()
    try:
        res = run_bass_kernel_spmd(nc, in_maps,
                                   core_ids=list(range(NCORES)), trace=trace)
    except Exception:
        if not trace:
            raise
        res = run_bass_kernel_spmd(nc, in_maps,
                                   core_ids=list(range(NCORES)), trace=False)
    _CACHE["last_result"] = res
    outs = []
    for c in range(NCORES):
        oT = res.results[c]["outT"]  # [2, 128, T] f32
        o = oT.reshape(DIM, T).T.reshape(BL, N, DIM)
        outs.append(o)
    return np.concatenate(outs, 0).astype(np.float32)
